# revision 4
# baseline (speedup 1.0000x reference)
"""Two-layer GAT (PyG GATConv semantics) on 8 Trainium2 NeuronCores.

Strategy (dst-sharded, duplicate-free scatter rounds):
  - Nodes are split into 8 contiguous ranges; core c owns dst nodes
    [c*NS, (c+1)*NS) and every edge whose dst falls in its range.
  - Per core, edges are grouped by (round, bank): `round` is the edge's
    rank within its dst segment (so every scatter call sees distinct dst
    rows -> no CCE read-modify-write races), `bank` is the src row-range
    quarter of the gather table (so int16 dma_gather indices stay < 32768).
  - Layer tables (h | fused attention scores) are built per-core with
    TensorE matmuls using host-folded weights, then AllGathered so each
    core can gather arbitrary src rows locally.
  - Per-edge work: dma_gather by src (h + sc_src), dma_gather by dst
    (sc_dst from the local slice), leaky-relu + exp on ACT/DVE, messages
    msg = exp(e) * h on DVE, dma_scatter_add into U = [sum msg | sum exp].
  - Softmax normalization happens per *node* (alpha = ex * (1/den)[dst]),
    so no per-edge division is needed for the aggregation itself; the
    alpha outputs are produced in a final pass that gathers [1/den1|1/den2]
    rows per edge and multiplies by the stashed exp values.
"""

import dataclasses
import os
import numpy as np

import concourse.bass as bass
import concourse.bacc as bacc
import concourse.mybir as mybir
from concourse import tile
from concourse.masks import make_identity
from concourse.bass_utils import run_bass_kernel_spmd

dt = mybir.dt
AF = mybir.ActivationFunctionType
ALU = mybir.AluOpType

NCORE = 8
NEG_SLOPE = 0.2
EPS = 1e-16
T_CH = 20          # max 128-edge chunks per edge tile
NT = 512           # node-phase matmul free-dim tile


def _bc(ap, dim, count):
    """Insert a stride-0 (broadcast) dim of size `count` at position `dim`."""
    new = [list(p) for p in ap.ap]
    new.insert(dim, [0, count])
    return dataclasses.replace(ap, ap=new)


# ----------------------------------------------------------------------------
# host-side preprocessing
# ----------------------------------------------------------------------------

def _plan(N, src, dst):
    NS = N // NCORE
    VR = ((NS + 127) // 128) * 128
    NBLK = VR // 128
    FT = NCORE * VR
    NBANK = 1
    while FT // NBANK > 32000:
        NBANK *= 2
    assert NBANK <= 8 and FT % NBANK == 0
    BANKR = FT // NBANK

    loops = np.arange(N, dtype=np.int64)
    s_all = np.concatenate([src.astype(np.int64), loops])
    d_all = np.concatenate([dst.astype(np.int64), loops])
    ET = len(s_all)

    gid = (s_all // NS) * VR + (s_all % NS)
    core_of = d_all // NS
    dloc_all = d_all % NS
    bank_all = gid // BANKR

    cores = []
    for c in range(NCORE):
        sel = np.flatnonzero(core_of == c)
        dl = dloc_all[sel]
        order = np.argsort(dl, kind="stable")
        sorted_d = dl[order]
        starts = np.r_[0, np.nonzero(np.diff(sorted_d))[0] + 1]
        counts = np.diff(np.r_[starts, len(sel)])
        within = np.arange(len(sel)) - np.repeat(starts, counts)
        rk = np.empty(len(sel), np.int64)
        rk[order] = within
        cores.append(dict(sel=sel, dl=dl, gi=gid[sel], bk=bank_all[sel], rk=rk))

    maxrk = int(max(p["rk"].max() for p in cores)) + 1

    segchunks = np.zeros((maxrk, NBANK), np.int64)
    for p in cores:
        key = p["rk"] * NBANK + p["bk"]
        cnt = np.bincount(key, minlength=maxrk * NBANK).reshape(maxrk, NBANK)
        segchunks = np.maximum(segchunks, (cnt + 127) // 128)

    tiles = []
    seg_ofs = {}
    off = 0
    for r in range(maxrk):
        for b in range(NBANK):
            ch = int(segchunks[r, b])
            if ch == 0:
                continue
            seg_ofs[(r, b)] = off
            done = 0
            while done < ch:
                cap = min(T_CH, ch - done)
                tiles.append((r, b, off + done * 128, cap))
                done += cap
            off += ch * 128
    L = off

    built = []
    for p in cores:
        key = p["rk"] * NBANK + p["bk"]
        order = np.argsort(key, kind="stable")
        ks = key[order]
        starts = np.r_[0, np.nonzero(np.diff(ks))[0] + 1]
        counts = np.diff(np.r_[starts, len(ks)])
        within = np.arange(len(ks)) - np.repeat(starts, counts)
        base = np.array([seg_ofs[(int(k) // NBANK, int(k) % NBANK)] for k in ks[starts]],
                        dtype=np.int64)
        slot_sorted = np.repeat(base, counts) + within
        slot = np.empty(len(ks), np.int64)
        slot[order] = slot_sorted
        gidx = np.zeros(L, np.int64)
        didx = np.full(L, VR, np.int64)
        gidx[slot] = p["gi"] - p["bk"] * BANKR
        didx[slot] = p["dl"]
        built.append(dict(slot=slot, gidx=gidx, didx=didx, sel=p["sel"]))

    cfg = dict(N=N, NS=NS, VR=VR, NBLK=NBLK, FT=FT, NBANK=NBANK, BANKR=BANKR,
               ET=ET, L=L, maxrk=maxrk, tiles=tiles)
    return cfg, built


def _pack16(v):
    a = v.reshape(-1, 16).T.astype(np.int16)
    return np.tile(a, (8, 1))


def _pack_idx_arrays(cfg, built):
    tiles = cfg["tiles"]
    out = []
    for b in built:
        gparts, dparts = [], []
        pos_of_slot = np.empty(cfg["L"], np.int64)
        base = 0
        for (_r, _bk, ofs, cap) in tiles:
            n = cap * 128
            gparts.append(_pack16(b["gidx"][ofs:ofs + n]).ravel())
            dparts.append(_pack16(b["didx"][ofs:ofs + n]).ravel())
            j = np.arange(n)
            pos_of_slot[ofs:ofs + n] = base + (j % 128) * cap + (j // 128)
            base += n
        out.append(dict(gflat=np.concatenate(gparts), dflat=np.concatenate(dparts),
                        pos_of_slot=pos_of_slot, slot=b["slot"], sel=b["sel"]))
    return out


# ----------------------------------------------------------------------------
# device program
# ----------------------------------------------------------------------------

def _build_program(cfg, HEADS, HID, NCLS, FIN):
    VR, NBLK, FT = cfg["VR"], cfg["NBLK"], cfg["FT"]
    BANKR = cfg["BANKR"]
    tiles = cfg["tiles"]
    F1 = HEADS * HID
    M1 = F1 + 2 * HEADS
    W1R = 128
    M2 = NCLS + 2
    W2R = 64
    KCH = (FIN + 127) // 128
    NCT = (VR + NT - 1) // NT
    NTP = NCT * NT
    ntot = sum(128 * cap for (_, _, _, cap) in tiles)
    idxtot = ntot * 8      # int16 entries per idx array (128 x cap*8 per tile)

    nc = bacc.Bacc("TRN2", target_bir_lowering=False, debug=False,
                   num_devices=NCORE)

    x_t = nc.declare_dram_parameter("x_t", [FIN, NTP], dt.float32, isOutput=False)
    w1 = nc.declare_dram_parameter("w1", [128, KCH * M1], dt.float32, isOutput=False)
    w2 = nc.declare_dram_parameter("w2", [F1, M2], dt.float32, isOutput=False)
    b1r = nc.declare_dram_parameter("b1r", [128, F1], dt.float32, isOutput=False)
    b2r = nc.declare_dram_parameter("b2r", [128, NCLS], dt.float32, isOutput=False)
    gidx_d = nc.declare_dram_parameter("gidx", [idxtot], dt.int16, isOutput=False)
    didx_d = nc.declare_dram_parameter("didx", [idxtot], dt.int16, isOutput=False)
    a1_d = nc.declare_dram_parameter("a1", [ntot * HEADS], dt.float32, isOutput=True)
    a2_d = nc.declare_dram_parameter("a2", [ntot], dt.float32, isOutput=True)
    logp_d = nc.declare_dram_parameter("logp", [VR, NCLS], dt.float32, isOutput=True)
    logits_d = nc.declare_dram_parameter("logits", [VR, NCLS], dt.float32, isOutput=True)
    U1_d = nc.declare_dram_parameter("U1", [VR + 1, W1R], dt.float32, isOutput=True)
    U2_d = nc.declare_dram_parameter("U2", [VR + 1, W2R], dt.float32, isOutput=True)

    t1loc = nc.dram_tensor("t1loc", [VR + 2, W1R], dt.float32)
    t1full = nc.dram_tensor("t1full", [FT, W1R], dt.float32, addr_space="Shared")
    t2loc = nc.dram_tensor("t2loc", [VR + 2, W2R], dt.float32)
    t2full = nc.dram_tensor("t2full", [FT, W2R], dt.float32, addr_space="Shared")
    ex1_d = nc.dram_tensor("ex1", [ntot * HEADS], dt.float32)
    ex2_d = nc.dram_tensor("ex2", [ntot], dt.float32)
    rden_d = nc.dram_tensor("rden", [VR + 2, W2R], dt.float32)

    with tile.TileContext(nc) as tc:
        with (tc.tile_pool(name="const", bufs=1) as constp,
              tc.tile_pool(name="big", bufs=1) as bigp,
              tc.tile_pool(name="work", bufs=2) as workp,
              tc.tile_pool(name="psum", bufs=2, space="PSUM") as psump):

            ident = constp.tile([128, 128], dt.float32)
            make_identity(nc, ident[:])
            zrow = constp.tile([128, W1R], dt.float32)
            nc.gpsimd.memset(zrow[:], 0.0)
            nc.sync.dma_start(out=t1loc[VR:VR + 2, :], in_=zrow[0:2, :])
            nc.sync.dma_start(out=t2loc[VR:VR + 2, :], in_=zrow[0:2, 0:W2R])
            nc.sync.dma_start(out=rden_d[VR:VR + 2, :], in_=zrow[0:2, 0:W2R])

            w1_sb = constp.tile([128, KCH * M1], dt.float32)
            nc.sync.dma_start(out=w1_sb[:], in_=w1[:])
            w2_sb = constp.tile([F1, M2], dt.float32)
            nc.sync.dma_start(out=w2_sb[:], in_=w2[:])
            b1_sb = constp.tile([128, F1], dt.float32)
            nc.sync.dma_start(out=b1_sb[:], in_=b1r[:])
            b2_sb = constp.tile([128, NCLS], dt.float32)
            nc.sync.dma_start(out=b2_sb[:], in_=b2r[:])

            # ============ stage A: layer-1 node phase ============
            tab1 = bigp.tile([128, NBLK * W1R], dt.float32, tag="big50")
            nc.gpsimd.memset(tab1[:], 0.0)
            for t in range(NCT):
                xt_sb = workp.tile([128, KCH * NT], dt.float32, tag="xt")
                for k in range(KCH):
                    nc.sync.dma_start(out=xt_sb[:, k * NT:(k + 1) * NT],
                                      in_=x_t[k * 128:(k + 1) * 128, t * NT:(t + 1) * NT])
                ps = psump.tile([M1, NT], dt.float32, space="PSUM", tag="ps")
                for k in range(KCH):
                    nc.tensor.matmul(ps[:], w1_sb[:, k * M1:(k + 1) * M1],
                                     xt_sb[:, k * NT:(k + 1) * NT],
                                     start=(k == 0), stop=(k == KCH - 1))
                o1_sb = workp.tile([M1, NT], dt.float32, tag="o1")
                nc.vector.tensor_copy(o1_sb[:], ps[:])
                for j in range(min(4, NBLK - 4 * t)):
                    pst = psump.tile([128, M1], dt.float32, space="PSUM", tag="pst")
                    nc.tensor.transpose(pst[:], o1_sb[0:M1, j * 128:(j + 1) * 128],
                                        ident[0:M1, 0:M1])
                    b = 4 * t + j
                    nc.vector.tensor_copy(tab1[:, b * W1R:b * W1R + M1], pst[:])
            nc.sync.dma_start(
                out=t1loc[0:VR, :].rearrange("(b p) w -> p b w", p=128),
                in_=tab1[:].rearrange("p (b w) -> p b w", w=W1R))
            nc.gpsimd.collective_compute(
                "AllGather", ALU.bypass, ins=[t1loc[0:VR, :]], outs=[t1full[:]],
                replica_groups=[list(range(NCORE))])

            # ============ edge phase (both layers) ============
            def edge_phase(tfull, tloc, rowW, sc_ofs, nheads, U_out, ex_dram):
                hid = sc_ofs // nheads
                gofs = 0
                eofs = 0
                for (r, bk, sofs, cap) in tiles:
                    n = cap * 128
                    iw = cap * 8
                    gi = workp.tile([128, iw], dt.int16, tag="gi")
                    nc.sync.dma_start(out=gi[:], in_=gidx_d[gofs:gofs + 128 * iw]
                                      .rearrange("(p w) -> p w", p=128))
                    di = workp.tile([128, iw], dt.int16, tag="di")
                    nc.sync.dma_start(out=di[:], in_=didx_d[gofs:gofs + 128 * iw]
                                      .rearrange("(p w) -> p w", p=128))
                    gofs += 128 * iw

                    pay = workp.tile([128, cap * rowW], dt.float32, tag="pay")
                    payv = pay[:].rearrange("p (k w) -> p k w", w=rowW)
                    nc.gpsimd.dma_gather(
                        out_ap=payv,
                        in_ap=tfull[bk * BANKR:(bk + 1) * BANKR, :],
                        idxs_ap=gi[:], num_idxs=n, num_idxs_reg=n,
                        elem_size=rowW, single_packet=False)
                    sd = workp.tile([128, cap * 64], dt.float32, tag="sd")
                    sdv = sd[:].rearrange("p (k w) -> p k w", w=64)
                    nc.gpsimd.dma_gather(
                        out_ap=sdv,
                        in_ap=dataclasses.replace(tloc[0:VR + 1, 0:64],
                                                  offset=sc_ofs + nheads),
                        idxs_ap=di[:], num_idxs=n, num_idxs_reg=n,
                        elem_size=64, elem_step=rowW, single_packet=False)
                    exv = payv[:, :, sc_ofs:sc_ofs + nheads]
                    nc.vector.tensor_tensor(out=exv, in0=exv,
                                            in1=sdv[:, :, 0:nheads], op=ALU.add)
                    nc.scalar.activation(exv, exv, AF.Prelu, alpha=NEG_SLOPE)
                    nc.scalar.activation(exv, exv, AF.Exp)
                    nc.sync.dma_start(
                        out=ex_dram[eofs:eofs + n * nheads]
                        .rearrange("(p k w) -> p k w", p=128, w=nheads),
                        in_=exv)
                    eofs += n * nheads
                    msg = payv[:, :, 0:sc_ofs].rearrange("p k (h c) -> p k h c", c=hid)
                    nc.vector.tensor_tensor(out=msg, in0=msg, in1=_bc(exv, 3, hid),
                                            op=ALU.mult)
                    nc.gpsimd.dma_scatter_add(
                        U_out[:], payv, di[:], n, n, rowW, single_packet=False)

            edge_phase(t1full, t1loc, W1R, F1, HEADS, U1_d, ex1_d)

            # ============ stage C: normalize L1, build L2 table ============
            U1s = bigp.tile([128, NBLK * W1R], dt.float32, tag="big50")
            nc.sync.dma_start(out=U1s[:].rearrange("p (b w) -> p b w", w=W1R),
                              in_=U1_d[0:VR, :].rearrange("(b p) w -> p b w", p=128))
            U1v = U1s[:].rearrange("p (b w) -> p b w", w=W1R)
            rd1 = bigp.tile([128, NBLK * HEADS], dt.float32, tag="rd1")
            rd1v = rd1[:].rearrange("p (b h) -> p b h", h=HEADS)
            scr = workp.tile([128, NBLK * HEADS], dt.float32, tag="scr")
            nc.vector.tensor_scalar_add(rd1v, U1v[:, :, F1:F1 + HEADS], EPS)
            nc.vector.reciprocal_approx_accurate(rd1[:], rd1[:], scr[:])
            nc.sync.dma_start(
                out=rden_d[0:VR, 0:HEADS].rearrange("(b p) w -> p b w", p=128),
                in_=rd1v)
            x1 = bigp.tile([128, NBLK * F1], dt.float32, tag="big25a")
            x1v = x1[:].rearrange("p (b w) -> p b w", w=F1)
            x1v4 = x1[:].rearrange("p (b h c) -> p b h c", h=HEADS, c=HID)
            nc.vector.tensor_tensor(out=x1v4,
                                    in0=U1v[:, :, 0:F1].rearrange("p b (h c) -> p b h c", c=HID),
                                    in1=_bc(rd1v, 3, HID), op=ALU.mult)
            nc.vector.tensor_tensor(out=x1v, in0=x1v, in1=_bc(b1_sb[:], 1, NBLK),
                                    op=ALU.add)
            xneg = bigp.tile([128, NBLK * F1], dt.float32, tag="big25b")
            nc.vector.tensor_scalar_min(xneg[:], x1[:], 0.0)
            nc.scalar.activation(xneg[:], xneg[:], AF.Exp)
            nc.vector.tensor_scalar_max(x1[:], x1[:], 0.0)
            nc.vector.tensor_tensor(out=x1[:], in0=x1[:], in1=xneg[:], op=ALU.add)
            nc.vector.tensor_scalar_add(x1[:], x1[:], -1.0)

            tab2 = bigp.tile([128, NBLK * W2R], dt.float32, tag="big25b")
            nc.gpsimd.memset(tab2[:], 0.0)
            for t in range(NCT):
                nb = min(4, NBLK - 4 * t)
                if nb <= 0:
                    break
                x1t_sb = workp.tile([F1, NT], dt.float32, tag="x1t")
                for j in range(nb):
                    ps = psump.tile([F1, 128], dt.float32, space="PSUM", tag="ps")
                    nc.tensor.transpose(ps[:], x1[:, (4 * t + j) * F1:(4 * t + j + 1) * F1],
                                        ident[:])
                    nc.vector.tensor_copy(x1t_sb[:, j * 128:(j + 1) * 128], ps[:])
                ps2 = psump.tile([M2, NT], dt.float32, space="PSUM", tag="ps")
                nc.tensor.matmul(ps2[:, 0:nb * 128], w2_sb[:], x1t_sb[:, 0:nb * 128],
                                 start=True, stop=True)
                o2_sb = workp.tile([M2, NT], dt.float32, tag="o2")
                nc.vector.tensor_copy(o2_sb[:, 0:nb * 128], ps2[:, 0:nb * 128])
                for j in range(nb):
                    pst = psump.tile([128, M2], dt.float32, space="PSUM", tag="pst")
                    nc.tensor.transpose(pst[:], o2_sb[0:M2, j * 128:(j + 1) * 128],
                                        ident[0:M2, 0:M2])
                    b = 4 * t + j
                    nc.vector.tensor_copy(tab2[:, b * W2R:b * W2R + M2], pst[:])
            nc.sync.dma_start(
                out=t2loc[0:VR, :].rearrange("(b p) w -> p b w", p=128),
                in_=tab2[:].rearrange("p (b w) -> p b w", w=W2R))
            nc.gpsimd.collective_compute(
                "AllGather", ALU.bypass, ins=[t2loc[0:VR, :]], outs=[t2full[:]],
                replica_groups=[list(range(NCORE))])

            # ============ stage D: layer-2 edge phase ============
            edge_phase(t2full, t2loc, W2R, NCLS, 1, U2_d, ex2_d)

            # ============ stage E: logits / logp ============
            U2s = bigp.tile([128, NBLK * W2R], dt.float32, tag="big50")
            nc.sync.dma_start(out=U2s[:].rearrange("p (b w) -> p b w", w=W2R),
                              in_=U2_d[0:VR, :].rearrange("(b p) w -> p b w", p=128))
            U2v = U2s[:].rearrange("p (b w) -> p b w", w=W2R)
            rd2 = workp.tile([128, NBLK], dt.float32, tag="rd2")
            rd2v = rd2[:].rearrange("p (b o) -> p b o", o=1)
            scr2 = workp.tile([128, NBLK], dt.float32, tag="scr2")
            nc.vector.tensor_scalar_add(rd2v, U2v[:, :, NCLS:NCLS + 1], EPS)
            nc.vector.reciprocal_approx_accurate(rd2[:], rd2[:], scr2[:])
            nc.sync.dma_start(
                out=rden_d[0:VR, HEADS:HEADS + 1].rearrange("(b p) w -> p b w", p=128),
                in_=rd2v)
            lg = bigp.tile([128, NBLK * NCLS], dt.float32, tag="big25a")
            lgv = lg[:].rearrange("p (b w) -> p b w", w=NCLS)
            nc.vector.tensor_tensor(out=lgv, in0=U2v[:, :, 0:NCLS],
                                    in1=_bc(rd2[:], 2, NCLS), op=ALU.mult)
            nc.vector.tensor_tensor(out=lgv, in0=lgv, in1=_bc(b2_sb[:], 1, NBLK),
                                    op=ALU.add)
            nc.sync.dma_start(out=logits_d[:].rearrange("(b p) w -> p b w", p=128),
                              in_=lgv)
            mx = workp.tile([128, NBLK], dt.float32, tag="mx")
            nc.vector.tensor_reduce(mx[:].rearrange("p (b o) -> p b o", o=1), lgv,
                                    axis=mybir.AxisListType.X, op=ALU.max)
            lp = bigp.tile([128, NBLK * NCLS], dt.float32, tag="big25b")
            lpv = lp[:].rearrange("p (b w) -> p b w", w=NCLS)
            nc.vector.tensor_tensor(out=lpv, in0=lgv, in1=_bc(mx[:], 2, NCLS),
                                    op=ALU.subtract)
            esum = workp.tile([128, NBLK * NCLS], dt.float32, tag="esum")
            nc.scalar.activation(esum[:], lp[:], AF.Exp)
            ssum = workp.tile([128, NBLK], dt.float32, tag="ssum")
            nc.vector.tensor_reduce(ssum[:].rearrange("p (b o) -> p b o", o=1),
                                    esum[:].rearrange("p (b w) -> p b w", w=NCLS),
                                    axis=mybir.AxisListType.X, op=ALU.add)
            nc.scalar.activation(ssum[:], ssum[:], AF.Ln)
            nc.vector.tensor_tensor(out=lpv, in0=lpv, in1=_bc(ssum[:], 2, NCLS),
                                    op=ALU.subtract)
            nc.sync.dma_start(out=logp_d[:].rearrange("(b p) w -> p b w", p=128),
                              in_=lpv)

            # ============ stage F: alpha outputs ============
            gofs = 0
            eofs1 = 0
            eofs2 = 0
            for (r, bk, sofs, cap) in tiles:
                n = cap * 128
                iw = cap * 8
                di = workp.tile([128, iw], dt.int16, tag="di")
                nc.sync.dma_start(out=di[:], in_=didx_d[gofs:gofs + 128 * iw]
                                  .rearrange("(p w) -> p w", p=128))
                gofs += 128 * iw
                rdn = workp.tile([128, cap * 64], dt.float32, tag="sd")
                rdnv = rdn[:].rearrange("p (k w) -> p k w", w=64)
                nc.gpsimd.dma_gather(
                    out_ap=rdnv, in_ap=rden_d[0:VR + 1, 0:64],
                    idxs_ap=di[:], num_idxs=n, num_idxs_reg=n,
                    elem_size=64, elem_step=W2R, single_packet=False)
                e1 = workp.tile([128, cap * HEADS], dt.float32, tag="fe1")
                e1v = e1[:].rearrange("p (k h) -> p k h", h=HEADS)
                nc.sync.dma_start(out=e1[:], in_=ex1_d[eofs1:eofs1 + n * HEADS]
                                  .rearrange("(p w) -> p w", p=128))
                nc.vector.tensor_tensor(out=e1v, in0=e1v, in1=rdnv[:, :, 0:HEADS],
                                        op=ALU.mult)
                nc.sync.dma_start(out=a1_d[eofs1:eofs1 + n * HEADS]
                                  .rearrange("(p w) -> p w", p=128), in_=e1[:])
                eofs1 += n * HEADS
                e2 = workp.tile([128, cap], dt.float32, tag="fe2")
                e2v = e2[:].rearrange("p (k o) -> p k o", o=1)
                nc.sync.dma_start(out=e2[:], in_=ex2_d[eofs2:eofs2 + n]
                                  .rearrange("(p w) -> p w", p=128))
                nc.vector.tensor_tensor(out=e2v, in0=e2v,
                                        in1=rdnv[:, :, HEADS:HEADS + 1], op=ALU.mult)
                nc.sync.dma_start(out=a2_d[eofs2:eofs2 + n]
                                  .rearrange("(p w) -> p w", p=128), in_=e2[:])
                eofs2 += n

    nc.compile()
    return nc


# ----------------------------------------------------------------------------
# public entry point
# ----------------------------------------------------------------------------

def kernel(x, src, dst, W1, a_src1, a_dst1, b1, W2, a_src2, a_dst2, b2):
    x = np.asarray(x, np.float32)
    src = np.asarray(src)
    dst = np.asarray(dst)
    W1 = np.asarray(W1, np.float32); a_src1 = np.asarray(a_src1, np.float32)
    a_dst1 = np.asarray(a_dst1, np.float32); b1 = np.asarray(b1, np.float32)
    W2 = np.asarray(W2, np.float32); a_src2 = np.asarray(a_src2, np.float32)
    a_dst2 = np.asarray(a_dst2, np.float32); b2 = np.asarray(b2, np.float32)

    N, FIN = x.shape
    HEADS, HID = a_src1.shape
    NCLS = W2.shape[1]
    F1 = HEADS * HID

    cfg, built = _plan(N, src, dst)
    packed = _pack_idx_arrays(cfg, built)
    NS, VR = cfg["NS"], cfg["VR"]
    nc = _build_program(cfg, HEADS, HID, NCLS, FIN)

    A1s = np.stack([W1[:, h * HID:(h + 1) * HID] @ a_src1[h] for h in range(HEADS)], 1)
    A1d = np.stack([W1[:, h * HID:(h + 1) * HID] @ a_dst1[h] for h in range(HEADS)], 1)
    lhsT1 = np.concatenate([W1, A1s, A1d], axis=1).astype(np.float32)
    KCH = (FIN + 127) // 128
    M1 = F1 + 2 * HEADS
    w1_in = np.zeros((128, KCH * M1), np.float32)
    for k in range(KCH):
        w1_in[:, k * M1:(k + 1) * M1] = lhsT1[k * 128:(k + 1) * 128]
    lhsT2 = np.concatenate([W2, W2 @ a_src2.T, W2 @ a_dst2.T], axis=1).astype(np.float32)
    b1rep = np.tile(b1[None, :], (128, 1)).astype(np.float32)
    b2rep = np.tile(b2[None, :], (128, 1)).astype(np.float32)

    NCT = (VR + NT - 1) // NT
    NTP = NCT * NT
    in_maps = []
    for c in range(NCORE):
        xt = np.zeros((FIN, NTP), np.float32)
        xt[:, 0:NS] = x[c * NS:(c + 1) * NS].T
        in_maps.append(dict(
            x_t=xt, w1=w1_in, w2=lhsT2, b1r=b1rep, b2r=b2rep,
            gidx=packed[c]["gflat"], didx=packed[c]["dflat"]))

    trace = bool(int(os.environ.get("GAT_TRACE", "0")))
    kres = run_bass_kernel_spmd(nc, in_maps, list(range(NCORE)), trace=trace)
    global LAST_RESULTS
    LAST_RESULTS = kres
    res = kres.results

    ET = cfg["ET"]
    logp = np.empty((N, NCLS), np.float32)
    logits = np.empty((N, NCLS), np.float32)
    alpha1 = np.empty((ET, HEADS), np.float32)
    alpha2 = np.empty((ET, 1), np.float32)
    for c in range(NCORE):
        r = res[c]
        logp[c * NS:(c + 1) * NS] = r["logp"][0:NS]
        logits[c * NS:(c + 1) * NS] = r["logits"][0:NS]
        pos = packed[c]["pos_of_slot"][packed[c]["slot"]]
        alpha1[packed[c]["sel"]] = r["a1"].reshape(-1, HEADS)[pos]
        alpha2[packed[c]["sel"], 0] = r["a2"][pos]
    return logp, alpha1, alpha2, logits
